# revision 12
# baseline (speedup 1.0000x reference)
"""Trainium2 Bass kernel for GTPCA topk_masking layer.

Computation (see reference):
  wn     = w / sqrt(sum(w^2)/n),  n = 128*128
  scores = valid_xcorr2d(inputs, wn) / n          -> (B, 113, 113)
  idx    = argmax |scores| (flat, first occurrence)
  out    = scores[idx] * wn placed as a 16x16 patch at idx, zeros elsewhere

Device strategy (pure data parallel over 8 cores, 512 images each):
  - The 2D correlation runs on the tensor engine in fp8e4 with
    perf_mode=DoubleRow: for each kernel-column pair (q, q+1) a Toeplitz
    stationary [128, 2, 113] (T_q | T_{q+1}) contracts against moving pairs
    (img col j+q, img col j+q+1), accumulating over the 8 pairs in PSUM.
    The pair stream comes from two SBUF copies of each image group, the
    second shifted left by one column.  DoubleRow streams 2 fp8 pairs per
    cycle, so each matmul costs ~0.5 cycles per output column -- ~4x fewer
    PE cycles than the fp32r 16-pass formulation.
  - Per PSUM bank (4 images) one fused DVE reduce with apply_absolute_value
    produces the per-row abs-max of the score map.
  - Only the per-row abs-max [113, 512] leaves the device.  The host takes
    candidate rows within CAND_TOL of each image's device max (fp8
    quantization shifts rowmax by <4% of the max; the 20% gate has >2x
    margin), rescores those rows exactly in fp64, picks the true argmax,
    and scatters smax*wn patches into the output.
"""

import sys

import numpy as np

if "/opt/trn_rl_repo" not in sys.path:
    sys.path.insert(0, "/opt/trn_rl_repo")

import ml_dtypes

FP8 = ml_dtypes.float8_e4m3

N_CORES = 8
B = 4096
H = W = 128
KH = KW = 16
SH = SW = H - KH + 1  # 113
SW_PAD = 113  # moving span per image
W_PAD = 130  # sbuf image width: cols up to 127+1 needed for the shifted copy
W_PADD = 132  # dram padded width
TT_PAD = 128  # ttoe innermost pad; DoubleRow k-tile stride must be %16==0
N_ELEM = H * W  # 16384
PER_CORE = B // N_CORES  # 512
GROUP = 32  # images per DMA/compute group
BANK = 4  # images per PSUM bank (4*114 = 456 <= 512 psum fp32 limit)
CAND_TOL = 0.12  # candidate-row gate vs device global max (3x the max observed
                 # fp8 rowmax deficit of 4% at the true argmax row)


MODE = "dr"  # "dr" = DoubleRow, "drsw" = DoubleRowSwInterleave
TT_SW_PAD = 240  # per-qp stride of interleaved weights (16-aligned)
LDW_OPT = False  # walrus ldw-opt rejects explicit InstLdweights; keep disabled.
# (Measured: the kernel is matmul-stream-bound, LDWEIGHTS is already hidden.)


def _patch_ldw_opt():
    """Flip walrus's --enable-ldw-opt to true: consecutive matmuls sharing a
    stationary then skip the redundant 226-column LDWEIGHTS reload, which is
    the serial bottleneck of the DoubleRow datapath."""
    import concourse.bass_utils as bu

    if getattr(bu, "_ldw_opt_patched", False):
        return
    orig = bu.run_command

    def run_command_ldw(cmd, *a, **k):
        if LDW_OPT and isinstance(cmd, list):
            cmd = [
                "--enable-ldw-opt=true" if c == "--enable-ldw-opt=false" else c
                for c in cmd
            ]
        return orig(cmd, *a, **k)

    bu.run_command = run_command_ldw
    bu._ldw_opt_patched = True


def _build_nc(n_imgs: int, repeat: int = 1, mode: str | None = None):
    from contextlib import ExitStack

    import concourse.bacc as bacc
    import concourse.mybir as mybir
    import concourse.tile as tile

    mode = MODE if mode is None else mode
    _patch_ldw_opt()
    f32 = mybir.dt.float32
    f8 = mybir.dt.float8e4

    nc = bacc.Bacc("TRN2", target_bir_lowering=False)
    imgs_d = nc.dram_tensor("imgs", [H, n_imgs, W_PADD], f8, kind="ExternalInput")
    if mode == "drsw":
        ttoe_d = nc.dram_tensor(
            "ttoe", [H, KW // 2, TT_SW_PAD], f8, kind="ExternalInput"
        )
        pm = mybir.MatmulPerfMode.DoubleRowSwInterleave
    else:
        ttoe_d = nc.dram_tensor("ttoe", [H, KW, TT_PAD], f8, kind="ExternalInput")
        pm = mybir.MatmulPerfMode.DoubleRow
    rm_d = nc.dram_tensor("rowmax", [SH, n_imgs], f32, kind="ExternalOutput")

    n_groups = n_imgs // GROUP
    banks_per_group = GROUP // BANK

    with ExitStack() as ctx:
        tc = ctx.enter_context(tile.TileContext(nc))
        consts = ctx.enter_context(tc.tile_pool(name="consts", bufs=1))
        imgp = ctx.enter_context(tc.tile_pool(name="imgp", bufs=3))
        accp = ctx.enter_context(tc.tile_pool(name="accp", bufs=1, space="PSUM"))
        stage = ctx.enter_context(tc.tile_pool(name="stage", bufs=1))

        ttoe_t = consts.tile(list(ttoe_d.shape), f8)
        nc.sync.dma_start(ttoe_t[:], ttoe_d[:])
        rm_all = stage.tile([SH, n_imgs], f32)

        for _rep in range(repeat):
          for g in range(n_groups):
            img_t = imgp.tile([H, 2, GROUP, W_PAD], f8)
            sl = slice(g * GROUP, (g + 1) * GROUP)
            nc.sync.dma_start(img_t[:, 0], imgs_d[:, sl, 0:W_PAD])
            nc.sync.dma_start(img_t[:, 1], imgs_d[:, sl, 1 : 1 + W_PAD])

            psums = [
                accp.tile([SH, BANK, SW_PAD], f32, name=f"acc{bk}", tag=f"acc{bk}")
                for bk in range(banks_per_group)
            ]
            for qp in range(KW // 2):
                if mode == "drsw":
                    lhs = ttoe_t[:, qp, 0 : 2 * SH]
                else:
                    lhs = ttoe_t[:, 2 * qp : 2 * qp + 2, 0:SH]
                for bk in range(banks_per_group):
                    rhs = img_t[
                        :, :, bk * BANK : (bk + 1) * BANK, 2 * qp : 2 * qp + SW_PAD
                    ]
                    nc.tensor.matmul(
                        psums[bk][:],
                        lhs,
                        rhs,
                        start=(qp == 0),
                        stop=(qp == KW // 2 - 1),
                        perf_mode=pm,
                        skip_group_check=True,
                    )
            for bk in range(banks_per_group):
                base = g * GROUP + bk * BANK
                nc.vector.tensor_reduce(
                    rm_all[:, base : base + BANK],
                    psums[bk][:, :, 0:SW],
                    axis=mybir.AxisListType.X,
                    op=mybir.AluOpType.max,
                    apply_absolute_value=True,
                )

        nc.sync.dma_start(rm_d[:], rm_all[:])

    nc.compile()
    return nc


_NC_CACHE: dict = {}


def _get_nc(n_imgs: int):
    key = (n_imgs, MODE)
    if key not in _NC_CACHE:
        _NC_CACHE[key] = _build_nc(n_imgs)
    return _NC_CACHE[key]


def _weights_f32(w: np.ndarray) -> np.ndarray:
    w32 = np.asarray(w, dtype=np.float32)
    ss = np.sum(w32 * w32, dtype=np.float32)
    denom = np.sqrt(ss / np.float32(N_ELEM))
    return (w32 / denom).astype(np.float32)


def _toeplitz(wn: np.ndarray) -> np.ndarray:
    wn8 = wn.astype(FP8)
    if MODE == "drsw":
        # DoubleRowSwInterleave layout: per qp, flat columns are
        # [A_{112}, B_{112}, ..., A_0, B_0] with A = T_{2qp}, B = T_{2qp+1},
        # where T_q[r, i] = wn[r-i, q].
        T = np.zeros((H, SH, KW), dtype=FP8)  # T[r, i, q]
        for i in range(SH):
            T[i : i + KH, i, :] = wn8
        ttoe = np.zeros((H, KW // 2, TT_SW_PAD), dtype=FP8)
        rev = np.arange(SH - 1, -1, -1)
        for qp in range(KW // 2):
            ttoe[:, qp, 0 : 2 * SH : 2] = T[:, rev, 2 * qp]
            ttoe[:, qp, 1 : 2 * SH : 2] = T[:, rev, 2 * qp + 1]
        return ttoe
    ttoe = np.zeros((H, KW, TT_PAD), dtype=FP8)
    for i in range(SH):
        ttoe[i : i + KH, :, i] = wn8
    return ttoe


def _host_imgs(inputs_np: np.ndarray) -> np.ndarray:
    """Full-batch DRAM staging: [H, nb, W_PADD] fp8."""
    nb = inputs_np.shape[0]
    host = np.zeros((H, nb, W_PADD), dtype=FP8)
    host[:, :, :W] = inputs_np.transpose(1, 0, 2).astype(FP8)
    return host


def _run_device(inputs_np: np.ndarray, ttoe: np.ndarray, trace: bool = False):
    from concourse.bass_utils import run_bass_kernel_spmd

    nc = _get_nc(PER_CORE)
    host_t = _host_imgs(inputs_np)
    in_maps = []
    for c in range(N_CORES):
        shard = np.ascontiguousarray(host_t[:, c * PER_CORE : (c + 1) * PER_CORE, :])
        in_maps.append({"imgs": shard, "ttoe": ttoe})
    res = run_bass_kernel_spmd(
        nc, in_maps, core_ids=list(range(N_CORES)), trace=trace
    )
    rm = np.concatenate([r["rowmax"] for r in res.results], axis=1)  # [113, B]
    return rm, res


def _finalize(inputs_np: np.ndarray, wn: np.ndarray, rm: np.ndarray) -> np.ndarray:
    """Host: candidate rows -> exact rescore -> argmax -> patch scatter."""
    nb = rm.shape[1]
    gm = rm.max(axis=0)  # [nb] device global abs-max per image
    thr = gm * (1.0 - CAND_TOL)
    cb, ci = np.nonzero((rm >= thr[None, :]).T)  # image ids, candidate rows

    # exact scores for each candidate row, fp64, via per-p Toeplitz gemms
    row_idx = ci[:, None] + np.arange(KH)[None, :]  # [C, 16]
    wn64 = wn.astype(np.float64)
    T = np.zeros((KH, W, SW), dtype=np.float64)  # T[p][col, j] = wn[p, col-j]
    for j in range(SW):
        T[:, j : j + KW, j] = wn64
    n_cand = len(cb)
    scores = np.empty((n_cand, SW), dtype=np.float64)
    chunk = 65536
    for s in range(0, n_cand, chunk):
        e = min(s + chunk, n_cand)
        strips = inputs_np[cb[s:e, None], row_idx[s:e], :].astype(np.float64)
        acc = np.zeros((e - s, SW), dtype=np.float64)
        for p in range(KH):
            acc += strips[:, p, :] @ T[p]
        scores[s:e] = acc
    scores /= float(N_ELEM)

    # per image: among candidate rows pick max |score|, ties -> lowest flat idx
    flat = ci[:, None].astype(np.int64) * SW + np.arange(SW)[None, :]
    abss = np.abs(scores)
    best_val = np.zeros(nb, dtype=np.float64)
    best_flat = np.zeros(nb, dtype=np.int64)
    best_abs = np.full(nb, -1.0, dtype=np.float64)
    # reduce per candidate-row first
    j_best = np.argmax(abss, axis=1)  # first occurrence within row
    r_abs = abss[np.arange(n_cand), j_best]
    r_val = scores[np.arange(n_cand), j_best]
    r_flat = flat[np.arange(n_cand), j_best]
    # then reduce across rows of the same image (first occurrence on exact ties)
    order = np.lexsort((r_flat, -r_abs, cb))  # grouped by image
    cb_o = cb[order]
    first = np.unique(cb_o, return_index=True)[1]
    sel = order[first]
    img_ids = cb[sel]
    best_val[img_ids] = r_val[sel]
    best_flat[img_ids] = r_flat[sel]
    best_abs[img_ids] = r_abs[sel]
    assert np.all(best_abs >= 0.0), "some image had no candidate rows"

    rows = (best_flat // SW).astype(np.int64)
    cols = (best_flat % SW).astype(np.int64)
    vals = best_val.astype(np.float32)

    out = np.zeros((nb, H, W), dtype=np.float32)
    patches = vals[:, None, None] * wn[None, :, :]  # [nb, 16, 16] f32
    bidx = np.arange(nb)[:, None, None]
    ridx = rows[:, None, None] + np.arange(KH)[None, :, None]
    cidx = cols[:, None, None] + np.arange(KW)[None, None, :]
    out[bidx, ridx, cidx] = patches
    return out


def kernel(inputs: np.ndarray, w: np.ndarray) -> np.ndarray:
    inputs_np = np.ascontiguousarray(np.asarray(inputs, dtype=np.float32))
    wn = _weights_f32(w)
    ttoe = _toeplitz(wn)
    rm, _ = _run_device(inputs_np, ttoe)
    return _finalize(inputs_np, wn, rm)


# revision 13
# speedup vs baseline: 1.3452x; 1.3452x over previous
"""Trainium2 Bass kernel for GTPCA topk_masking layer.

Computation (see reference):
  wn     = w / sqrt(sum(w^2)/n),  n = 128*128
  scores = valid_xcorr2d(inputs, wn) / n          -> (B, 113, 113)
  idx    = argmax |scores| (flat, first occurrence)
  out    = scores[idx] * wn placed as a 16x16 patch at idx, zeros elsewhere

Device strategy (pure data parallel over 8 cores, 512 images each):
  - The 2D correlation runs on the tensor engine in fp8e4 with
    perf_mode=DoubleRow: for each kernel-column pair (q, q+1) a Toeplitz
    stationary [128, 2, 113] (T_q | T_{q+1}) contracts against moving pairs
    (img col j+q, img col j+q+1), accumulating over the 8 pairs in PSUM.
    The pair stream comes from two SBUF copies of each image group, the
    second shifted left by one column.  DoubleRow streams 2 fp8 pairs per
    cycle, so each matmul costs ~0.5 cycles per output column -- ~4x fewer
    PE cycles than the fp32r 16-pass formulation.
  - Per PSUM bank (4 images) one fused DVE reduce with apply_absolute_value
    produces the per-row abs-max of the score map.
  - Only the per-row abs-max [113, 512] leaves the device.  The host takes
    candidate rows within CAND_TOL of each image's device max (fp8
    quantization shifts rowmax by <4% of the max; the 20% gate has >2x
    margin), rescores those rows exactly in fp64, picks the true argmax,
    and scatters smax*wn patches into the output.
"""

import sys

import numpy as np

if "/opt/trn_rl_repo" not in sys.path:
    sys.path.insert(0, "/opt/trn_rl_repo")

import ml_dtypes

FP8 = ml_dtypes.float8_e4m3

N_CORES = 8
B = 4096
H = W = 128
KH = KW = 16
SH = SW = H - KH + 1  # 113
SW_PAD = 113  # moving span per image
W_PAD = 130  # sbuf image width: cols up to 127+1 needed for the shifted copy
W_PADD = 132  # dram padded width
TT_PAD = 128  # ttoe innermost pad; DoubleRow k-tile stride must be %16==0
N_ELEM = H * W  # 16384
PER_CORE = B // N_CORES  # 512
GROUP = 16  # images per DMA/compute group
BANK = 4  # images per PSUM bank (4*114 = 456 <= 512 psum fp32 limit)
CAND_TOL = 0.12  # candidate-row gate vs device global max (3x the max observed
                 # fp8 rowmax deficit of 4% at the true argmax row)


MODE = "dr"  # "dr" = DoubleRow, "drsw" = DoubleRowSwInterleave
TT_SW_PAD = 240  # per-qp stride of interleaved weights (16-aligned)
LDW_OPT = False  # walrus ldw-opt rejects explicit InstLdweights; keep disabled.
# (Measured: the kernel is matmul-stream-bound, LDWEIGHTS is already hidden.)


def _patch_ldw_opt():
    """Flip walrus's --enable-ldw-opt to true: consecutive matmuls sharing a
    stationary then skip the redundant 226-column LDWEIGHTS reload, which is
    the serial bottleneck of the DoubleRow datapath."""
    import concourse.bass_utils as bu

    if getattr(bu, "_ldw_opt_patched", False):
        return
    orig = bu.run_command

    def run_command_ldw(cmd, *a, **k):
        if LDW_OPT and isinstance(cmd, list):
            cmd = [
                "--enable-ldw-opt=true" if c == "--enable-ldw-opt=false" else c
                for c in cmd
            ]
        return orig(cmd, *a, **k)

    bu.run_command = run_command_ldw
    bu._ldw_opt_patched = True


def _build_nc(n_imgs: int, repeat: int = 1, mode: str | None = None):
    from contextlib import ExitStack

    import concourse.bacc as bacc
    import concourse.mybir as mybir
    import concourse.tile as tile

    mode = MODE if mode is None else mode
    _patch_ldw_opt()
    f32 = mybir.dt.float32
    f8 = mybir.dt.float8e4

    nc = bacc.Bacc("TRN2", target_bir_lowering=False)
    imgs_d = nc.dram_tensor("imgs", [H, n_imgs, W_PADD], f8, kind="ExternalInput")
    if mode == "drsw":
        ttoe_d = nc.dram_tensor(
            "ttoe", [H, KW // 2, TT_SW_PAD], f8, kind="ExternalInput"
        )
        pm = mybir.MatmulPerfMode.DoubleRowSwInterleave
    else:
        ttoe_d = nc.dram_tensor("ttoe", [H, KW, TT_PAD], f8, kind="ExternalInput")
        pm = mybir.MatmulPerfMode.DoubleRow
    rm_d = nc.dram_tensor("rowmax", [SH, n_imgs], f32, kind="ExternalOutput")

    n_groups = n_imgs // GROUP
    banks_per_group = GROUP // BANK

    with ExitStack() as ctx:
        tc = ctx.enter_context(tile.TileContext(nc))
        consts = ctx.enter_context(tc.tile_pool(name="consts", bufs=1))
        imgp = ctx.enter_context(tc.tile_pool(name="imgp", bufs=3))
        accp = ctx.enter_context(tc.tile_pool(name="accp", bufs=2, space="PSUM"))
        stage = ctx.enter_context(tc.tile_pool(name="stage", bufs=1))

        ttoe_t = consts.tile(list(ttoe_d.shape), f8)
        nc.sync.dma_start(ttoe_t[:], ttoe_d[:])
        rm_all = stage.tile([SH, n_imgs], f32)

        for _rep in range(repeat):
          for g in range(n_groups):
            img_t = imgp.tile([H, 2, GROUP, W_PAD], f8)
            sl = slice(g * GROUP, (g + 1) * GROUP)
            nc.sync.dma_start(img_t[:, 0], imgs_d[:, sl, 0:W_PAD])
            nc.sync.dma_start(img_t[:, 1], imgs_d[:, sl, 1 : 1 + W_PAD])

            psums = [
                accp.tile([SH, BANK, SW_PAD], f32, name=f"acc{bk}", tag=f"acc{bk}")
                for bk in range(banks_per_group)
            ]
            for qp in range(KW // 2):
                if mode == "drsw":
                    lhs = ttoe_t[:, qp, 0 : 2 * SH]
                else:
                    lhs = ttoe_t[:, 2 * qp : 2 * qp + 2, 0:SH]
                for bk in range(banks_per_group):
                    rhs = img_t[
                        :, :, bk * BANK : (bk + 1) * BANK, 2 * qp : 2 * qp + SW_PAD
                    ]
                    nc.tensor.matmul(
                        psums[bk][:],
                        lhs,
                        rhs,
                        start=(qp == 0),
                        stop=(qp == KW // 2 - 1),
                        perf_mode=pm,
                        skip_group_check=True,
                    )
            for bk in range(banks_per_group):
                base = g * GROUP + bk * BANK
                nc.vector.tensor_reduce(
                    rm_all[:, base : base + BANK],
                    psums[bk][:, :, 0:SW],
                    axis=mybir.AxisListType.X,
                    op=mybir.AluOpType.max,
                    apply_absolute_value=True,
                )

        nc.sync.dma_start(rm_d[:], rm_all[:])

    nc.compile()
    return nc


_NC_CACHE: dict = {}


def _get_nc(n_imgs: int):
    key = (n_imgs, MODE)
    if key not in _NC_CACHE:
        _NC_CACHE[key] = _build_nc(n_imgs)
    return _NC_CACHE[key]


def _weights_f32(w: np.ndarray) -> np.ndarray:
    w32 = np.asarray(w, dtype=np.float32)
    ss = np.sum(w32 * w32, dtype=np.float32)
    denom = np.sqrt(ss / np.float32(N_ELEM))
    return (w32 / denom).astype(np.float32)


def _toeplitz(wn: np.ndarray) -> np.ndarray:
    wn8 = wn.astype(FP8)
    if MODE == "drsw":
        # DoubleRowSwInterleave layout: per qp, flat columns are
        # [A_{112}, B_{112}, ..., A_0, B_0] with A = T_{2qp}, B = T_{2qp+1},
        # where T_q[r, i] = wn[r-i, q].
        T = np.zeros((H, SH, KW), dtype=FP8)  # T[r, i, q]
        for i in range(SH):
            T[i : i + KH, i, :] = wn8
        ttoe = np.zeros((H, KW // 2, TT_SW_PAD), dtype=FP8)
        rev = np.arange(SH - 1, -1, -1)
        for qp in range(KW // 2):
            ttoe[:, qp, 0 : 2 * SH : 2] = T[:, rev, 2 * qp]
            ttoe[:, qp, 1 : 2 * SH : 2] = T[:, rev, 2 * qp + 1]
        return ttoe
    ttoe = np.zeros((H, KW, TT_PAD), dtype=FP8)
    for i in range(SH):
        ttoe[i : i + KH, :, i] = wn8
    return ttoe


def _host_imgs(inputs_np: np.ndarray) -> np.ndarray:
    """Full-batch DRAM staging: [H, nb, W_PADD] fp8."""
    nb = inputs_np.shape[0]
    host = np.zeros((H, nb, W_PADD), dtype=FP8)
    host[:, :, :W] = inputs_np.transpose(1, 0, 2).astype(FP8)
    return host


def _run_device(inputs_np: np.ndarray, ttoe: np.ndarray, trace: bool = False):
    from concourse.bass_utils import run_bass_kernel_spmd

    nc = _get_nc(PER_CORE)
    host_t = _host_imgs(inputs_np)
    in_maps = []
    for c in range(N_CORES):
        shard = np.ascontiguousarray(host_t[:, c * PER_CORE : (c + 1) * PER_CORE, :])
        in_maps.append({"imgs": shard, "ttoe": ttoe})
    res = run_bass_kernel_spmd(
        nc, in_maps, core_ids=list(range(N_CORES)), trace=trace
    )
    rm = np.concatenate([r["rowmax"] for r in res.results], axis=1)  # [113, B]
    return rm, res


def _finalize(inputs_np: np.ndarray, wn: np.ndarray, rm: np.ndarray) -> np.ndarray:
    """Host: candidate rows -> exact rescore -> argmax -> patch scatter."""
    nb = rm.shape[1]
    gm = rm.max(axis=0)  # [nb] device global abs-max per image
    thr = gm * (1.0 - CAND_TOL)
    cb, ci = np.nonzero((rm >= thr[None, :]).T)  # image ids, candidate rows

    # exact scores for each candidate row, fp64, via per-p Toeplitz gemms
    row_idx = ci[:, None] + np.arange(KH)[None, :]  # [C, 16]
    wn64 = wn.astype(np.float64)
    T = np.zeros((KH, W, SW), dtype=np.float64)  # T[p][col, j] = wn[p, col-j]
    for j in range(SW):
        T[:, j : j + KW, j] = wn64
    n_cand = len(cb)
    scores = np.empty((n_cand, SW), dtype=np.float64)
    chunk = 65536
    for s in range(0, n_cand, chunk):
        e = min(s + chunk, n_cand)
        strips = inputs_np[cb[s:e, None], row_idx[s:e], :].astype(np.float64)
        acc = np.zeros((e - s, SW), dtype=np.float64)
        for p in range(KH):
            acc += strips[:, p, :] @ T[p]
        scores[s:e] = acc
    scores /= float(N_ELEM)

    # per image: among candidate rows pick max |score|, ties -> lowest flat idx
    flat = ci[:, None].astype(np.int64) * SW + np.arange(SW)[None, :]
    abss = np.abs(scores)
    best_val = np.zeros(nb, dtype=np.float64)
    best_flat = np.zeros(nb, dtype=np.int64)
    best_abs = np.full(nb, -1.0, dtype=np.float64)
    # reduce per candidate-row first
    j_best = np.argmax(abss, axis=1)  # first occurrence within row
    r_abs = abss[np.arange(n_cand), j_best]
    r_val = scores[np.arange(n_cand), j_best]
    r_flat = flat[np.arange(n_cand), j_best]
    # then reduce across rows of the same image (first occurrence on exact ties)
    order = np.lexsort((r_flat, -r_abs, cb))  # grouped by image
    cb_o = cb[order]
    first = np.unique(cb_o, return_index=True)[1]
    sel = order[first]
    img_ids = cb[sel]
    best_val[img_ids] = r_val[sel]
    best_flat[img_ids] = r_flat[sel]
    best_abs[img_ids] = r_abs[sel]
    assert np.all(best_abs >= 0.0), "some image had no candidate rows"

    rows = (best_flat // SW).astype(np.int64)
    cols = (best_flat % SW).astype(np.int64)
    vals = best_val.astype(np.float32)

    out = np.zeros((nb, H, W), dtype=np.float32)
    patches = vals[:, None, None] * wn[None, :, :]  # [nb, 16, 16] f32
    bidx = np.arange(nb)[:, None, None]
    ridx = rows[:, None, None] + np.arange(KH)[None, :, None]
    cidx = cols[:, None, None] + np.arange(KW)[None, None, :]
    out[bidx, ridx, cidx] = patches
    return out


def kernel(inputs: np.ndarray, w: np.ndarray) -> np.ndarray:
    inputs_np = np.ascontiguousarray(np.asarray(inputs, dtype=np.float32))
    wn = _weights_f32(w)
    ttoe = _toeplitz(wn)
    rm, _ = _run_device(inputs_np, ttoe)
    return _finalize(inputs_np, wn, rm)
